# revision 33
# baseline (speedup 1.0000x reference)
"""AspectMatcher distributed Trainium2 kernel.

reference math (B=32, S=1024, H=1024):
    q      = seq @ W.T + b          # [B,S,H]
    logits = q @ seq.T (per batch)  # [B,S,S]
    loss   = masked-mean CE of log_softmax(logits) at tgt indices

Sharding: data-parallel over batch, 4 batches per core on 8 NeuronCores.
Per batch A = seq[b]: logits = A W^T A^T + (A b) broadcast.  Both matmuls
contract A over its feature axis, so the device only needs A^T (features on
partitions); the host passes seq pre-transposed (and bf16-cast) per batch.
The device produces full fp32 logits plus per-row log-sum-exp (softmax
statistics fused on the PSUM tiles); the scalar loss is assembled on host
from logits[b,s,tgt] - lse (O(B*S) work).
"""

import numpy as np
import ml_dtypes
from contextlib import ExitStack

import concourse.bass as bass
import concourse.tile as tile
from concourse import bacc, mybir
from concourse.bass_utils import run_bass_kernel_spmd

B, S, H = 32, 1024, 1024
NCORES = 8
BPC = B // NCORES          # batches per core
P = 128
KH = H // P                # contraction blocks (h)
MS = S // P                # output row blocks (s)
FD = 512                   # matmul moving free dim (one PSUM bank)
NT = S // FD               # free-dim chunks per row block

BF16 = mybir.dt.bfloat16
F32 = mybir.dt.float32
AF = mybir.ActivationFunctionType
AX = mybir.AxisListType
OP = mybir.AluOpType

_CACHED_NC = None


def build_nc():
    nc = bacc.Bacc("TRN2", target_bir_lowering=False, debug=False,
                   num_devices=NCORES)

    seqT = nc.dram_tensor("seqT", [BPC, H, S], BF16, kind="ExternalInput").ap()
    # W swizzled host-side into mo-major slabs [mo, kh, p, c] so one 256KB
    # contiguous slab covers everything output-block mo needs
    wTs = nc.dram_tensor("wTs", [KH, KH, P, P], BF16, kind="ExternalInput").ap()
    biasP = nc.dram_tensor("biasP", [P, KH], F32, kind="ExternalInput").ap()
    logits = nc.dram_tensor("logits", [BPC, S, S], F32,
                            kind="ExternalOutput").ap()
    # stats stored partition-major ([b, p, ms]) so the DMA is a plain 2D
    # contiguous transfer on the HW DGE; host reorders to [b, ms*P+p]
    mrow = nc.dram_tensor("mrow", [BPC, P, MS], F32, kind="ExternalOutput").ap()
    sexp = nc.dram_tensor("sexp", [BPC, P, MS], F32, kind="ExternalOutput").ap()

    with tile.TileContext(nc) as tc, ExitStack() as ctx:
        const = ctx.enter_context(tc.tile_pool(name="const", bufs=1))
        at_pool = ctx.enter_context(tc.tile_pool(name="at", bufs=3))
        qt_pool = ctx.enter_context(tc.tile_pool(name="qt", bufs=2))
        lg_pool = ctx.enter_context(tc.tile_pool(name="lg", bufs=4))
        ex_pool = ctx.enter_context(tc.tile_pool(name="ex", bufs=2))
        st_pool = ctx.enter_context(tc.tile_pool(name="st", bufs=12))
        ps_q = ctx.enter_context(tc.tile_pool(name="psq", bufs=2, space="PSUM"))
        ps_l = ctx.enter_context(tc.tile_pool(name="psl", bufs=4, space="PSUM"))

        # Startup-critical bytes = at(0) + the first W slab only.  The mo=0
        # slab goes out first on the scalar queue, at(0) chunks alternate
        # across both queues, and the remaining W slabs stream behind them.
        wt_sb = const.tile([P, KH, KH, P], BF16)   # [p, mo, kh, c]
        bias_sb = const.tile([P, KH], F32)
        nc.sync.dma_start(bias_sb[:], biasP[:])
        nc.scalar.dma_start(wt_sb[:, 0], wTs[0].rearrange("kh p c -> p kh c"))

        for b in range(BPC):
            at = at_pool.tile([P, KH, S], BF16)
            seqT_r = seqT[b].rearrange("(kh p) s -> p kh s", p=P)
            for kh in range(KH):
                if b == 0:
                    (nc.sync if kh % 2 == 0 else nc.scalar).dma_start(
                        at[:, kh, :], seqT_r[:, kh, :])
                else:
                    nc.sync.dma_start(at[:, kh, :], seqT_r[:, kh, :])
            if b == 0:
                for mo in range(1, KH):
                    nc.scalar.dma_start(
                        wt_sb[:, mo], wTs[mo].rearrange("kh p c -> p kh c"))

            # mm1: qT[o, s] = sum_h wT[h, o] * aT[h, s]   (+ bias on evict)
            qt = qt_pool.tile([P, KH, S], BF16)
            for mo in range(KH):
                for ns in range(NT):
                    pq = ps_q.tile([P, FD], F32)
                    for kh in range(KH):
                        nc.tensor.matmul(
                            pq[:],
                            wt_sb[:, mo, kh, :],
                            at[:, kh, ns * FD:(ns + 1) * FD],
                            start=(kh == 0), stop=(kh == KH - 1),
                        )
                    nc.vector.tensor_scalar_add(
                        qt[:, mo, ns * FD:(ns + 1) * FD], pq[:],
                        bias_sb[:, mo:mo + 1])

            # mm2: logits[s, t] = sum_o qT[o, s] * aT[o, t], fused softmax stats
            m_b = st_pool.tile([P, MS], F32, name="m_b")
            se_b = st_pool.tile([P, MS], F32, name="se_b")
            for ms in range(MS):
                lg = lg_pool.tile([P, S], F32)
                rmax = st_pool.tile([P, NT], F32, name="rmax")
                for nt in range(NT):
                    pl = ps_l.tile([P, FD], F32)
                    for ko in range(KH):
                        nc.tensor.matmul(
                            pl[:],
                            qt[:, ko, ms * P:(ms + 1) * P],
                            at[:, ko, nt * FD:(nt + 1) * FD],
                            start=(ko == 0), stop=(ko == KH - 1),
                        )
                    nc.scalar.activation(
                        lg[:, nt * FD:(nt + 1) * FD], pl[:], AF.Copy)
                    nc.vector.tensor_reduce(
                        rmax[:, nt:nt + 1], pl[:], axis=AX.X, op=OP.max)

                nc.vector.tensor_max(m_b[:, ms:ms + 1], rmax[:, 0:1], rmax[:, 1:2])
                # The last batch's exp+sum chains would back up ScalarE/VectorE
                # behind the final matmuls — skip them on device and
                # reconstruct those rows' sum-exp on host from the logits.
                if b == BPC - 1:
                    nc.vector.memset(se_b[:, ms:ms + 1], 1.0)
                else:
                    negm = st_pool.tile([P, 1], F32, name="negm")
                    nc.vector.tensor_scalar_mul(negm[:], m_b[:, ms:ms + 1], -1.0)
                    e = ex_pool.tile([P, S], F32)
                    nc.scalar.activation(e[:], lg[:], AF.Exp, bias=negm[:],
                                         scale=1.0)
                    nc.vector.tensor_reduce(
                        se_b[:, ms:ms + 1], e[:], axis=AX.X, op=OP.add)

                # logits out: split by partition halves across both DMA queues
                nc.sync.dma_start(
                    logits[b, ms * P:ms * P + 64, :], lg[:64, :])
                nc.scalar.dma_start(
                    logits[b, ms * P + 64:(ms + 1) * P, :], lg[64:, :])

            # batch epilogue: ship softmax stats; host computes lse = m + ln(se)
            nc.sync.dma_start(mrow[b], m_b[:])
            nc.scalar.dma_start(sexp[b], se_b[:])

    nc.compile()
    return nc


def _get_nc():
    global _CACHED_NC
    if _CACHED_NC is None:
        _CACHED_NC = build_nc()
    return _CACHED_NC


def run_device(seqT_bf, wTs_bf, biasP, trace=False, trace_kwargs=None):
    """seqT_bf: [B, H, S] bf16; wTs_bf: [KH, KH, P, P] bf16; biasP: [P, KH]."""
    nc = _get_nc()
    in_maps = []
    for c in range(NCORES):
        in_maps.append({
            "seqT": np.ascontiguousarray(seqT_bf[c * BPC:(c + 1) * BPC]),
            "wTs": wTs_bf,
            "biasP": biasP,
        })
    try:
        res = run_bass_kernel_spmd(
            nc, in_maps, core_ids=list(range(NCORES)),
            trace=trace, **(trace_kwargs or {}))
    except Exception:
        # transient device wedges (NRT_EXEC_UNIT_UNRECOVERABLE) usually
        # clear on the next attempt
        import time
        time.sleep(5)
        res = run_bass_kernel_spmd(
            nc, in_maps, core_ids=list(range(NCORES)),
            trace=trace, **(trace_kwargs or {}))
    logits = np.concatenate([res.results[c]["logits"] for c in range(NCORES)], 0)
    m = np.concatenate([res.results[c]["mrow"] for c in range(NCORES)], 0)
    se = np.concatenate([res.results[c]["sexp"] for c in range(NCORES)], 0)
    m = m.transpose(0, 2, 1).reshape(B, S)       # [b, p, ms] -> [b, ms*P+p]
    se = se.transpose(0, 2, 1).reshape(B, S)
    # each core's last batch had its sum-exp skipped on device; rebuild here
    for g in range(BPC - 1, B, BPC):
        se[g] = np.exp(logits[g] - m[g, :, None]).sum(axis=-1)
    lse = m + np.log(se)
    return logits, lse, res


def kernel(sequence_output, opinion_mask, tgt_asp_indices, W, b):
    seq = np.asarray(sequence_output, dtype=np.float32)
    W = np.asarray(W, dtype=np.float32)
    bias = np.asarray(b, dtype=np.float32)

    seqT_bf = seq.transpose(0, 2, 1).astype(ml_dtypes.bfloat16)
    # [mo, kh, p, c] with entry = W[mo*P+c, kh*P+p]
    wTs_bf = np.ascontiguousarray(
        W.reshape(KH, P, KH, P).transpose(0, 2, 3, 1)).astype(ml_dtypes.bfloat16)
    biasP = np.ascontiguousarray(bias.reshape(KH, P).T)

    logits, lse, _ = run_device(seqT_bf, wTs_bf, biasP)

    tgt = np.asarray(tgt_asp_indices).astype(np.int64)
    bi = np.arange(B)[:, None]
    si = np.arange(S)[None, :]
    tgt_lp = logits[bi, si, tgt] - lse                     # [B, S]
    mask = (np.asarray(opinion_mask) != 0).astype(np.float32)
    n = max(float(mask.sum()), 1.0)
    loss = np.float32(-(tgt_lp * mask).sum() / n)
    return loss, logits


# revision 35
# speedup vs baseline: 1.0144x; 1.0144x over previous
"""AspectMatcher distributed Trainium2 kernel.

reference math (B=32, S=1024, H=1024):
    q      = seq @ W.T + b          # [B,S,H]
    logits = q @ seq.T (per batch)  # [B,S,S]
    loss   = masked-mean CE of log_softmax(logits) at tgt indices

Sharding: data-parallel over batch, 4 batches per core on 8 NeuronCores.
Per batch A = seq[b]: logits = A W^T A^T + (A b) broadcast.  Both matmuls
contract A over its feature axis, so the device only needs A^T (features on
partitions); the host passes seq pre-transposed (and bf16-cast) per batch.
The device produces full fp32 logits plus per-row log-sum-exp (softmax
statistics fused on the PSUM tiles); the scalar loss is assembled on host
from logits[b,s,tgt] - lse (O(B*S) work).
"""

import numpy as np
import ml_dtypes
from contextlib import ExitStack

import concourse.bass as bass
import concourse.tile as tile
from concourse import bacc, mybir
from concourse.bass_utils import run_bass_kernel_spmd

B, S, H = 32, 1024, 1024
NCORES = 8
BPC = B // NCORES          # batches per core
P = 128
KH = H // P                # contraction blocks (h)
MS = S // P                # output row blocks (s)
FD = 512                   # matmul moving free dim (one PSUM bank)
NT = S // FD               # free-dim chunks per row block

BF16 = mybir.dt.bfloat16
F32 = mybir.dt.float32
AF = mybir.ActivationFunctionType
AX = mybir.AxisListType
OP = mybir.AluOpType

_CACHED_NC = None


def build_nc():
    nc = bacc.Bacc("TRN2", target_bir_lowering=False, debug=False,
                   num_devices=NCORES)

    seqT = nc.dram_tensor("seqT", [BPC, H, S], BF16, kind="ExternalInput").ap()
    # W swizzled host-side into mo-major slabs [mo, kh, p, c] so one 256KB
    # contiguous slab covers everything output-block mo needs
    wTs = nc.dram_tensor("wTs", [KH, KH, P, P], BF16, kind="ExternalInput").ap()
    biasP = nc.dram_tensor("biasP", [P, KH], F32, kind="ExternalInput").ap()
    logits = nc.dram_tensor("logits", [BPC, S, S], F32,
                            kind="ExternalOutput").ap()
    # stats stored partition-major ([b, p, ms]) so the DMA is a plain 2D
    # contiguous transfer on the HW DGE; host reorders to [b, ms*P+p]
    mrow = nc.dram_tensor("mrow", [BPC, P, MS], F32, kind="ExternalOutput").ap()
    sexp = nc.dram_tensor("sexp", [BPC, P, MS], F32, kind="ExternalOutput").ap()

    with tile.TileContext(nc) as tc, ExitStack() as ctx:
        const = ctx.enter_context(tc.tile_pool(name="const", bufs=1))
        at_pool = ctx.enter_context(tc.tile_pool(name="at", bufs=3))
        qt_pool = ctx.enter_context(tc.tile_pool(name="qt", bufs=2))
        lg_pool = ctx.enter_context(tc.tile_pool(name="lg", bufs=4))
        ex_pool = ctx.enter_context(tc.tile_pool(name="ex", bufs=2))
        st_pool = ctx.enter_context(tc.tile_pool(name="st", bufs=12))
        ps_q = ctx.enter_context(tc.tile_pool(name="psq", bufs=2, space="PSUM"))
        ps_l = ctx.enter_context(tc.tile_pool(name="psl", bufs=4, space="PSUM"))

        # Startup-critical bytes = at(0) + the first W slab only.  The mo=0
        # slab goes out first on the scalar queue, at(0) chunks alternate
        # across both queues, and the remaining W slabs stream behind them.
        wt_sb = const.tile([P, KH, KH, P], BF16)   # [p, mo, kh, c]
        bias_sb = const.tile([P, KH], F32)
        nc.sync.dma_start(bias_sb[:], biasP[:])
        nc.scalar.dma_start(wt_sb[:, 0], wTs[0].rearrange("kh p c -> p kh c"))

        for b in range(BPC):
            at = at_pool.tile([P, KH, S], BF16)
            seqT_r = seqT[b].rearrange("(kh p) s -> p kh s", p=P)
            for kh in range(KH):
                if b == 0:
                    (nc.sync if kh % 2 == 0 else nc.scalar).dma_start(
                        at[:, kh, :], seqT_r[:, kh, :])
                else:
                    nc.sync.dma_start(at[:, kh, :], seqT_r[:, kh, :])
            if b == 0:
                for mo in range(1, KH):
                    nc.scalar.dma_start(
                        wt_sb[:, mo], wTs[mo].rearrange("kh p c -> p kh c"))

            # mm1: qT[o, s] = sum_h wT[h, o] * aT[h, s]   (+ bias on evict)
            qt = qt_pool.tile([P, KH, S], BF16)
            for mo in range(KH):
                for ns in range(NT):
                    pq = ps_q.tile([P, FD], F32)
                    for kh in range(KH):
                        nc.tensor.matmul(
                            pq[:],
                            wt_sb[:, mo, kh, :],
                            at[:, kh, ns * FD:(ns + 1) * FD],
                            start=(kh == 0), stop=(kh == KH - 1),
                        )
                    nc.vector.tensor_scalar_add(
                        qt[:, mo, ns * FD:(ns + 1) * FD], pq[:],
                        bias_sb[:, mo:mo + 1])

            # mm2: logits[s, t] = sum_o qT[o, s] * aT[o, t], fused softmax stats
            m_b = st_pool.tile([P, MS], F32, name="m_b")
            se_b = st_pool.tile([P, MS], F32, name="se_b")
            for ms in range(MS):
                lg = lg_pool.tile([P, S], F32)
                rmax = st_pool.tile([P, NT], F32, name="rmax")
                for nt in range(NT):
                    pl = ps_l.tile([P, FD], F32)
                    for ko in range(KH):
                        nc.tensor.matmul(
                            pl[:],
                            qt[:, ko, ms * P:(ms + 1) * P],
                            at[:, ko, nt * FD:(nt + 1) * FD],
                            start=(ko == 0), stop=(ko == KH - 1),
                        )
                    nc.scalar.activation(
                        lg[:, nt * FD:(nt + 1) * FD], pl[:], AF.Copy)
                    nc.vector.tensor_reduce(
                        rmax[:, nt:nt + 1], pl[:], axis=AX.X, op=OP.max)

                nc.vector.tensor_max(m_b[:, ms:ms + 1], rmax[:, 0:1], rmax[:, 1:2])
                # The last batch's exp+sum chains would back up ScalarE/VectorE
                # behind the final matmuls — skip them on device and
                # reconstruct those rows' sum-exp on host from the logits.
                if b == BPC - 1:
                    nc.vector.memset(se_b[:, ms:ms + 1], 1.0)
                else:
                    negm = st_pool.tile([P, 1], F32, name="negm")
                    nc.vector.tensor_scalar_mul(negm[:], m_b[:, ms:ms + 1], -1.0)
                    e = ex_pool.tile([P, S], F32)
                    nc.scalar.activation(e[:], lg[:], AF.Exp, bias=negm[:],
                                         scale=1.0)
                    nc.vector.tensor_reduce(
                        se_b[:, ms:ms + 1], e[:], axis=AX.X, op=OP.add)

                # Outputs must stay off the sync queue while input prefetch is
                # live: the queue's cumulative completion semaphore would make
                # at(b+1)-ready waits count pending logits writes too (false
                # dependency, one lost MM slot ~4x/batch).  Only the last
                # batch (no loads left) splits across both queues for latency.
                if b < BPC - 1:
                    nc.scalar.dma_start(logits[b, ms * P:(ms + 1) * P, :], lg[:])
                else:
                    nc.sync.dma_start(
                        logits[b, ms * P:ms * P + 64, :], lg[:64, :])
                    nc.scalar.dma_start(
                        logits[b, ms * P + 64:(ms + 1) * P, :], lg[64:, :])

            # batch epilogue: ship softmax stats; host computes lse = m + ln(se)
            (nc.scalar if b < BPC - 1 else nc.sync).dma_start(mrow[b], m_b[:])
            nc.scalar.dma_start(sexp[b], se_b[:])

    nc.compile()
    return nc


def _get_nc():
    global _CACHED_NC
    if _CACHED_NC is None:
        _CACHED_NC = build_nc()
    return _CACHED_NC


def run_device(seqT_bf, wTs_bf, biasP, trace=False, trace_kwargs=None):
    """seqT_bf: [B, H, S] bf16; wTs_bf: [KH, KH, P, P] bf16; biasP: [P, KH]."""
    nc = _get_nc()
    in_maps = []
    for c in range(NCORES):
        in_maps.append({
            "seqT": np.ascontiguousarray(seqT_bf[c * BPC:(c + 1) * BPC]),
            "wTs": wTs_bf,
            "biasP": biasP,
        })
    try:
        res = run_bass_kernel_spmd(
            nc, in_maps, core_ids=list(range(NCORES)),
            trace=trace, **(trace_kwargs or {}))
    except Exception:
        # transient device wedges (NRT_EXEC_UNIT_UNRECOVERABLE) usually
        # clear on the next attempt
        import time
        time.sleep(5)
        res = run_bass_kernel_spmd(
            nc, in_maps, core_ids=list(range(NCORES)),
            trace=trace, **(trace_kwargs or {}))
    logits = np.concatenate([res.results[c]["logits"] for c in range(NCORES)], 0)
    m = np.concatenate([res.results[c]["mrow"] for c in range(NCORES)], 0)
    se = np.concatenate([res.results[c]["sexp"] for c in range(NCORES)], 0)
    m = m.transpose(0, 2, 1).reshape(B, S)       # [b, p, ms] -> [b, ms*P+p]
    se = se.transpose(0, 2, 1).reshape(B, S)
    # each core's last batch had its sum-exp skipped on device; rebuild here
    for g in range(BPC - 1, B, BPC):
        se[g] = np.exp(logits[g] - m[g, :, None]).sum(axis=-1)
    lse = m + np.log(se)
    return logits, lse, res


def kernel(sequence_output, opinion_mask, tgt_asp_indices, W, b):
    seq = np.asarray(sequence_output, dtype=np.float32)
    W = np.asarray(W, dtype=np.float32)
    bias = np.asarray(b, dtype=np.float32)

    seqT_bf = seq.transpose(0, 2, 1).astype(ml_dtypes.bfloat16)
    # [mo, kh, p, c] with entry = W[mo*P+c, kh*P+p]
    wTs_bf = np.ascontiguousarray(
        W.reshape(KH, P, KH, P).transpose(0, 2, 3, 1)).astype(ml_dtypes.bfloat16)
    biasP = np.ascontiguousarray(bias.reshape(KH, P).T)

    logits, lse, _ = run_device(seqT_bf, wTs_bf, biasP)

    tgt = np.asarray(tgt_asp_indices).astype(np.int64)
    bi = np.arange(B)[:, None]
    si = np.arange(S)[None, :]
    tgt_lp = logits[bi, si, tgt] - lse                     # [B, S]
    mask = (np.asarray(opinion_mask) != 0).astype(np.float32)
    n = max(float(mask.sum()), 1.0)
    loss = np.float32(-(tgt_lp * mask).sum() / n)
    return loss, logits


# revision 36
# speedup vs baseline: 1.2058x; 1.1887x over previous
"""AspectMatcher distributed Trainium2 kernel.

reference math (B=32, S=1024, H=1024):
    q      = seq @ W.T + b          # [B,S,H]
    logits = q @ seq.T (per batch)  # [B,S,S]
    loss   = masked-mean CE of log_softmax(logits) at tgt indices

Sharding: data-parallel over batch, 4 batches per core on 8 NeuronCores.
Per batch A = seq[b]: logits = A W^T A^T + (A b) broadcast.  Both matmuls
contract A over its feature axis, so the device only needs A^T (features on
partitions); the host passes seq pre-transposed (and bf16-cast) per batch.
The device produces full fp32 logits plus per-row log-sum-exp (softmax
statistics fused on the PSUM tiles); the scalar loss is assembled on host
from logits[b,s,tgt] - lse (O(B*S) work).
"""

import numpy as np
import ml_dtypes
from contextlib import ExitStack

import concourse.bass as bass
import concourse.tile as tile
from concourse import bacc, mybir
from concourse.bass_utils import run_bass_kernel_spmd

B, S, H = 32, 1024, 1024
NCORES = 8
BPC = B // NCORES          # batches per core
P = 128
KH = H // P                # contraction blocks (h)
MS = S // P                # output row blocks (s)
FD = 512                   # matmul moving free dim (one PSUM bank)
NT = S // FD               # free-dim chunks per row block

BF16 = mybir.dt.bfloat16
F32 = mybir.dt.float32
AF = mybir.ActivationFunctionType
AX = mybir.AxisListType
OP = mybir.AluOpType

_CACHED_NC = None


def build_nc():
    nc = bacc.Bacc("TRN2", target_bir_lowering=False, debug=False,
                   num_devices=NCORES)

    seqT = nc.dram_tensor("seqT", [BPC, H, S], BF16, kind="ExternalInput").ap()
    # W swizzled host-side into mo-major slabs [mo, kh, p, c] so one 256KB
    # contiguous slab covers everything output-block mo needs
    wTs = nc.dram_tensor("wTs", [KH, KH, P, P], BF16, kind="ExternalInput").ap()
    biasP = nc.dram_tensor("biasP", [P, KH], F32, kind="ExternalInput").ap()
    logits = nc.dram_tensor("logits", [BPC, S, S], F32,
                            kind="ExternalOutput").ap()
    # stats stored partition-major ([b, p, ms]) so the DMA is a plain 2D
    # contiguous transfer on the HW DGE; host reorders to [b, ms*P+p]
    mrow = nc.dram_tensor("mrow", [BPC, P, MS], F32, kind="ExternalOutput").ap()
    sexp = nc.dram_tensor("sexp", [BPC, P, MS], F32, kind="ExternalOutput").ap()

    with tile.TileContext(nc) as tc, ExitStack() as ctx:
        const = ctx.enter_context(tc.tile_pool(name="const", bufs=1))
        at_pool = ctx.enter_context(tc.tile_pool(name="at", bufs=3))
        qt_pool = ctx.enter_context(tc.tile_pool(name="qt", bufs=2))
        lg_pool = ctx.enter_context(tc.tile_pool(name="lg", bufs=4))
        ex_pool = ctx.enter_context(tc.tile_pool(name="ex", bufs=2))
        st_pool = ctx.enter_context(tc.tile_pool(name="st", bufs=12))
        ps_q = ctx.enter_context(tc.tile_pool(name="psq", bufs=3, space="PSUM"))
        ps_l = ctx.enter_context(tc.tile_pool(name="psl", bufs=4, space="PSUM"))

        # Startup-critical bytes = at(0) + the first W slab only.  The mo=0
        # slab goes out first on the scalar queue, at(0) chunks alternate
        # across both queues, and the remaining W slabs stream behind them.
        wt_sb = const.tile([P, KH, KH, P], BF16)   # [p, mo, kh, c]
        bias_sb = const.tile([P, KH], F32)
        nc.sync.dma_start(bias_sb[:], biasP[:])
        nc.scalar.dma_start(wt_sb[:, 0], wTs[0].rearrange("kh p c -> p kh c"))

        for b in range(BPC):
            at = at_pool.tile([P, KH, S], BF16)
            seqT_r = seqT[b].rearrange("(kh p) s -> p kh s", p=P)
            for kh in range(KH):
                if b == 0:
                    (nc.sync if kh % 2 == 0 else nc.scalar).dma_start(
                        at[:, kh, :], seqT_r[:, kh, :])
                else:
                    nc.sync.dma_start(at[:, kh, :], seqT_r[:, kh, :])
            if b == 0:
                for mo in range(1, KH):
                    nc.scalar.dma_start(
                        wt_sb[:, mo], wTs[mo].rearrange("kh p c -> p kh c"))

            # mm1: qT[o, s] = sum_h wT[h, o] * aT[h, s]   (+ bias on evict)
            qt = qt_pool.tile([P, KH, S], BF16)
            for mo in range(KH):
                for ns in range(NT):
                    pq = ps_q.tile([P, FD], F32)
                    for kh in range(KH):
                        nc.tensor.matmul(
                            pq[:],
                            wt_sb[:, mo, kh, :],
                            at[:, kh, ns * FD:(ns + 1) * FD],
                            start=(kh == 0), stop=(kh == KH - 1),
                        )
                    nc.vector.tensor_scalar_add(
                        qt[:, mo, ns * FD:(ns + 1) * FD], pq[:],
                        bias_sb[:, mo:mo + 1])

            # mm2: logits[s, t] = sum_o qT[o, s] * aT[o, t], fused softmax stats
            m_b = st_pool.tile([P, MS], F32, name="m_b")
            se_b = st_pool.tile([P, MS], F32, name="se_b")
            for ms in range(MS):
                lg = lg_pool.tile([P, S], F32)
                rmax = st_pool.tile([P, NT], F32, name="rmax")
                for nt in range(NT):
                    pl = ps_l.tile([P, FD], F32)
                    for ko in range(KH):
                        nc.tensor.matmul(
                            pl[:],
                            qt[:, ko, ms * P:(ms + 1) * P],
                            at[:, ko, nt * FD:(nt + 1) * FD],
                            start=(ko == 0), stop=(ko == KH - 1),
                        )
                    nc.scalar.activation(
                        lg[:, nt * FD:(nt + 1) * FD], pl[:], AF.Copy)
                    nc.vector.tensor_reduce(
                        rmax[:, nt:nt + 1], pl[:], axis=AX.X, op=OP.max)

                nc.vector.tensor_max(m_b[:, ms:ms + 1], rmax[:, 0:1], rmax[:, 1:2])
                # The last batch's exp+sum chains would back up ScalarE/VectorE
                # behind the final matmuls — skip them on device and
                # reconstruct those rows' sum-exp on host from the logits.
                if b == BPC - 1:
                    nc.vector.memset(se_b[:, ms:ms + 1], 1.0)
                else:
                    negm = st_pool.tile([P, 1], F32, name="negm")
                    nc.vector.tensor_scalar_mul(negm[:], m_b[:, ms:ms + 1], -1.0)
                    e = ex_pool.tile([P, S], F32)
                    nc.scalar.activation(e[:], lg[:], AF.Exp, bias=negm[:],
                                         scale=1.0)
                    nc.vector.tensor_reduce(
                        se_b[:, ms:ms + 1], e[:], axis=AX.X, op=OP.add)

                # Outputs must stay off the sync queue while input prefetch is
                # live: the queue's cumulative completion semaphore would make
                # at(b+1)-ready waits count pending logits writes too (false
                # dependency, one lost MM slot ~4x/batch).  Only the last
                # batch (no loads left) splits across both queues for latency.
                if b < BPC - 1:
                    nc.scalar.dma_start(logits[b, ms * P:(ms + 1) * P, :], lg[:])
                else:
                    nc.sync.dma_start(
                        logits[b, ms * P:ms * P + 64, :], lg[:64, :])
                    nc.scalar.dma_start(
                        logits[b, ms * P + 64:(ms + 1) * P, :], lg[64:, :])

            # batch epilogue: ship softmax stats; host computes lse = m + ln(se)
            (nc.scalar if b < BPC - 1 else nc.sync).dma_start(mrow[b], m_b[:])
            nc.scalar.dma_start(sexp[b], se_b[:])

    nc.compile()
    return nc


def _get_nc():
    global _CACHED_NC
    if _CACHED_NC is None:
        _CACHED_NC = build_nc()
    return _CACHED_NC


def run_device(seqT_bf, wTs_bf, biasP, trace=False, trace_kwargs=None):
    """seqT_bf: [B, H, S] bf16; wTs_bf: [KH, KH, P, P] bf16; biasP: [P, KH]."""
    nc = _get_nc()
    in_maps = []
    for c in range(NCORES):
        in_maps.append({
            "seqT": np.ascontiguousarray(seqT_bf[c * BPC:(c + 1) * BPC]),
            "wTs": wTs_bf,
            "biasP": biasP,
        })
    try:
        res = run_bass_kernel_spmd(
            nc, in_maps, core_ids=list(range(NCORES)),
            trace=trace, **(trace_kwargs or {}))
    except Exception:
        # transient device wedges (NRT_EXEC_UNIT_UNRECOVERABLE) usually
        # clear on the next attempt
        import time
        time.sleep(5)
        res = run_bass_kernel_spmd(
            nc, in_maps, core_ids=list(range(NCORES)),
            trace=trace, **(trace_kwargs or {}))
    logits = np.concatenate([res.results[c]["logits"] for c in range(NCORES)], 0)
    m = np.concatenate([res.results[c]["mrow"] for c in range(NCORES)], 0)
    se = np.concatenate([res.results[c]["sexp"] for c in range(NCORES)], 0)
    m = m.transpose(0, 2, 1).reshape(B, S)       # [b, p, ms] -> [b, ms*P+p]
    se = se.transpose(0, 2, 1).reshape(B, S)
    # each core's last batch had its sum-exp skipped on device; rebuild here
    for g in range(BPC - 1, B, BPC):
        se[g] = np.exp(logits[g] - m[g, :, None]).sum(axis=-1)
    lse = m + np.log(se)
    return logits, lse, res


def kernel(sequence_output, opinion_mask, tgt_asp_indices, W, b):
    seq = np.asarray(sequence_output, dtype=np.float32)
    W = np.asarray(W, dtype=np.float32)
    bias = np.asarray(b, dtype=np.float32)

    seqT_bf = seq.transpose(0, 2, 1).astype(ml_dtypes.bfloat16)
    # [mo, kh, p, c] with entry = W[mo*P+c, kh*P+p]
    wTs_bf = np.ascontiguousarray(
        W.reshape(KH, P, KH, P).transpose(0, 2, 3, 1)).astype(ml_dtypes.bfloat16)
    biasP = np.ascontiguousarray(bias.reshape(KH, P).T)

    logits, lse, _ = run_device(seqT_bf, wTs_bf, biasP)

    tgt = np.asarray(tgt_asp_indices).astype(np.int64)
    bi = np.arange(B)[:, None]
    si = np.arange(S)[None, :]
    tgt_lp = logits[bi, si, tgt] - lse                     # [B, S]
    mask = (np.asarray(opinion_mask) != 0).astype(np.float32)
    n = max(float(mask.sum()), 1.0)
    loss = np.float32(-(tgt_lp * mask).sum() / n)
    return loss, logits
